# revision 1
# baseline (speedup 1.0000x reference)
"""Chebyshev approximation kernel for Trainium2 (8 NeuronCores, SPMD data-parallel).

Math: reference computes
    y_at_nodes = (1-t) * y[:, idx] + t * y[:, idx+1]      # [n_obs, deg]
    out        = (y_at_nodes @ basis).reshape(-1)         # [n_obs*deg]
Both steps are linear in y, so we fold them into a single matrix on host:
    C[k, d] = sum_j W[k, j] * basis[j, d],   W = interp weights (2 nnz/col)
    out     = y @ C          # [n_obs, 2049] @ [2049, 1024]
The device kernel is one GEMM per 128-row block: PE-transpose the y block
(grid axis onto partitions, float32r transpose-mode matmuls into PSUM, drained
by wide DVE/ACT copies), then 16 accumulating float32r matmuls (k-tiles of
128) per 512-wide output half; grid column 2048's rank-1 contribution is
folded on DVE during the output copy. float32r = fp32 storage with FP22
multiplies at full PE rate (1 cycle/row for N>=256), fp32 accumulation.

Sharding: y rows split 8192/core across 8 cores; C replicated.
"""

import os
import numpy as np

DEG = 1024
N_OBS = 65536
M_P1 = 2049
N_CORES = 8
ROWS_PER_CORE = N_OBS // N_CORES  # 8192
KT = 17                           # contraction tiles of 128 (2049 -> 2176 padded)
KP = KT * 128                     # 2176
RB = 128                          # rows per block

_COMPILED = {}
LAST_RESULTS = None


def _cheb_c_matrix(x: np.ndarray) -> np.ndarray:
    """C [KP, DEG] float32 with zero pad rows >= 2049; out = y @ C[:M_P1]."""
    x = np.asarray(x, dtype=np.float32)
    k = np.arange(DEG, dtype=np.float32)
    # float32 node computation, mimicking the jax reference
    ang = (np.float32(np.pi) * (k + np.float32(0.5))) / np.float32(DEG)
    nodes = np.sort(np.cos(ang.astype(np.float32)).astype(np.float32))
    norm = ((np.float32(2.0) - (k == 0).astype(np.float32)) / np.float32(DEG)).astype(
        np.float64
    )
    # basis[j, d] = norm_d * cos(d * arccos(node_j)); f64 from f32 nodes
    theta = np.arccos(nodes.astype(np.float64))
    basis = norm[None, :] * np.cos(k.astype(np.float64)[None, :] * theta[:, None])
    idx = np.clip(np.searchsorted(x, nodes, side="right") - 1, 0, M_P1 - 2)
    a = x[idx]
    b = x[idx + 1]
    t = ((nodes - a) / (b - a)).astype(np.float64)
    C = np.zeros((KP, DEG), dtype=np.float64)
    np.add.at(C, idx, (1.0 - t)[:, None] * basis)
    np.add.at(C, idx + 1, t[:, None] * basis)
    return np.ascontiguousarray(C.astype(np.float32))


def build_cheb_kernel(tc, y_ap, c_ap, id_ap, o_ap, rows):
    """Emit the per-core program: out[rows, DEG] = y[rows, M_P1] @ C[:M_P1]."""
    import concourse.mybir as mybir

    nc = tc.nc
    f32 = mybir.dt.float32
    f32r = mybir.dt.float32r
    nblocks = rows // RB

    # 16 full k-tiles cover columns 0..2047; column 2048's rank-1 update is
    # folded on DVE during the output copy (out += y[:,2048] * C[2048,:]).
    KTM = 16
    # Stages grouped 4-per-PSUM-bank: a burst of 4 PE transposes shares one
    # PSUM bank, drained by a single wide copy; main matmuls run one group
    # behind so the drain is off their critical path.
    G = 4

    with (
        tc.tile_pool(name="consts", bufs=1) as consts,
        tc.tile_pool(name="ypool", bufs=4) as ypool,
        tc.tile_pool(name="ytpool", bufs=2) as ytpool,
        tc.tile_pool(name="opool", bufs=3) as opool,
        tc.tile_pool(name="pst", bufs=4, space="PSUM") as pstp,
        tc.tile_pool(name="pso", bufs=2, space="PSUM") as psop,
    ):
        ident = consts.tile([128, 128], f32r)
        nc.sync.dma_start(out=ident, in_=id_ap)
        # C resident in SBUF: [partition-within-tile, ktile, d]; chunked DMAs
        # on the scalar HWDGE queue so y loads (sync queue) aren't blocked.
        # Alternate C chunks between the scalar and sync HWDGE queues:
        # serialized on one queue the 16 chunks take ~24us and the first
        # blocks' matmuls stall waiting for late k-tiles. (gpsimd SWDGE is
        # avoided — its ring setup adds ~5us to engine startup.)
        c_sb = consts.tile([128, KTM, DEG], f32r)
        c_r = c_ap.rearrange("(t p) n -> p t n", p=128)
        def load_c(k):
            eng = nc.scalar if k % 2 == 0 else nc.sync
            eng.dma_start(out=c_sb[:, k, :], in_=c_r[:, k, :])
        # C row 2048 replicated across partitions for the DVE rank-1 fold.
        c_rep = consts.tile([128, DEG], f32)
        import concourse.bass as bass

        c_row = c_ap[KTM * 128 : KTM * 128 + 1, :].bitcast(f32)
        c_row_bc = bass.AP(
            tensor=c_row.tensor, offset=c_row.offset, ap=[[0, 128]] + list(c_row.ap[1:])
        )

        ybs, ytbs, pss = {}, {}, {}

        def load_y(b, split=False):
            yb = ypool.tile([128, M_P1], f32r, name="yb", tag="yb")
            rows = y_ap[b * RB : (b + 1) * RB, :]
            if split:
                # halves so block 0's first transposes start sooner
                nc.sync.dma_start(out=yb[:, 0:1024], in_=rows[:, 0:1024])
                nc.sync.dma_start(out=yb[:, 1024:M_P1], in_=rows[:, 1024:M_P1])
            else:
                nc.sync.dma_start(out=yb, in_=rows)
            ybs[b] = yb

        def emit_t_group(b, g):
            if g == 0:
                ytbs[b] = ytpool.tile([128, KTM, 128], f32r, name="ytb", tag="ytb")
            pst = pstp.tile([128, G, 128], f32r, name="pst", tag="pst")
            for j in range(G):
                k = g * G + j
                nc.tensor.transpose(
                    pst[:, j, :], ybs[b][:, k * 128 : (k + 1) * 128], ident
                )
            dst = ytbs[b][:, g * G : (g + 1) * G, :]
            if g % 2 == 0:
                nc.vector.tensor_copy(dst, pst)
            else:
                nc.scalar.copy(dst, pst)

        def emit_m_group(b, g):
            if g == 0:
                pss[b] = psop.tile([128, DEG], f32, name="ps", tag="ps")
            ps = pss[b]
            for j in range(G):
                k = g * G + j
                for nh in range(2):
                    nc.tensor.matmul(
                        ps[:, nh * 512 : (nh + 1) * 512],
                        ytbs[b][:, k, :],
                        c_sb[:, k, nh * 512 : (nh + 1) * 512],
                        start=(k == 0),
                        stop=(k == KTM - 1),
                    )
            if g == KTM // G - 1:
                tmp = opool.tile([128, DEG], f32, name="tmp", tag="tmp")
                nc.vector.tensor_scalar_mul(
                    tmp, c_rep, ybs[b][:, 2048:2049].bitcast(f32)
                )
                osb = opool.tile([128, DEG], f32, name="osb", tag="osb")
                nc.vector.tensor_add(osb, ps, tmp)
                nc.scalar.dma_start(out=o_ap[b * RB : (b + 1) * RB, :], in_=osb)
                del ybs[b], ytbs[b], pss[b]

        groups = [(b, g) for b in range(nblocks) for g in range(KTM // G)]
        load_y(0, split=True)
        for k in range(KTM):
            load_c(k)
        nc.scalar.dma_start(out=c_rep, in_=c_row_bc)
        for i in range(len(groups) + 1):
            if i < len(groups):
                b, g = groups[i]
                if g == 0 and b + 1 < nblocks:
                    load_y(b + 1)
                emit_t_group(b, g)
            if i >= 1:
                emit_m_group(*groups[i - 1])


def _build_nc(rows):
    import concourse.mybir as mybir
    import concourse.tile as tile
    from concourse import bacc

    f32 = mybir.dt.float32
    f32r = mybir.dt.float32r
    nc = bacc.Bacc(
        "TRN2",
        target_bir_lowering=False,
        debug=False,
        enable_asserts=False,
        num_devices=N_CORES,
    )
    y_ap = nc.dram_tensor("y", [rows, M_P1], f32r, kind="ExternalInput").ap()
    c_ap = nc.dram_tensor("c", [KP, DEG], f32r, kind="ExternalInput").ap()
    id_ap = nc.dram_tensor("ident", [128, 128], f32r, kind="ExternalInput").ap()
    o_ap = nc.dram_tensor("o", [rows, DEG], f32, kind="ExternalOutput").ap()
    with tile.TileContext(nc) as tc:
        build_cheb_kernel(tc, y_ap, c_ap, id_ap, o_ap, rows)
    nc.compile()
    return nc


def _get_compiled(rows=ROWS_PER_CORE):
    if rows not in _COMPILED:
        _COMPILED[rows] = _build_nc(rows)
    return _COMPILED[rows]


def kernel(x: np.ndarray, y: np.ndarray) -> np.ndarray:
    global LAST_RESULTS
    from concourse import bass_utils

    x = np.asarray(x, dtype=np.float32)
    y = np.ascontiguousarray(np.asarray(y, dtype=np.float32))
    assert y.shape == (N_OBS, M_P1), y.shape
    C = _cheb_c_matrix(x)

    nc = _get_compiled()
    ident = np.ascontiguousarray(np.eye(128, dtype=np.float32))
    in_maps = [
        {"y": y[i * ROWS_PER_CORE : (i + 1) * ROWS_PER_CORE], "c": C, "ident": ident}
        for i in range(N_CORES)
    ]
    trace = bool(int(os.environ.get("CHEB_TRACE", "0")))
    res = bass_utils.run_bass_kernel_spmd(
        nc, in_maps, core_ids=list(range(N_CORES)), trace=trace
    )
    LAST_RESULTS = res
    out = np.concatenate([res.results[i]["o"] for i in range(N_CORES)], axis=0)
    return out.reshape(-1)



# revision 3
# speedup vs baseline: 10249.5529x; 10249.5529x over previous
"""Chebyshev approximation kernel for Trainium2 (8 NeuronCores, SPMD data-parallel).

Math: reference computes
    z   = (1-t) * y[:, idx] + t * y[:, idx+1]   # interp at nodes [n_obs, deg]
    out = (z @ basis).reshape(-1)               # [n_obs*deg]

Device pipeline (per 128-row block, y carried in bf16):
  1. DMA the y block (bf16, padded to 2052 cols) into SBUF.
  2. DVE copies a 1-element-shifted bf16 view next to it, so that for ANY
     index i the pair (y[i], y[i+1]) lives in a single aligned 4-byte word:
     even i -> word i/2 of the raw copy, odd i -> word (i-1)/2 of the
     shifted copy.
  3. GPSIMD ap_gather fetches one word per Chebyshev node (1024 idxs).
  4. DVE lerp: t = pairs * [1-t_d, t_d] (elementwise), z_d = t[2d]+t[2d+1].
  5. PE transposes z (8x bf16 128x128) and runs the k=1024 GEMM z @ basis
     (16 accumulating bf16 matmuls into PSUM).
  6. ACT drains PSUM; result written back as bf16 and upcast on host.

Interp weights/indices and the basis are computed on host from x (tiny).
Sharding: y rows split 8192/core across 8 cores; consts replicated.
"""

import os
import numpy as np

DEG = 1024
N_OBS = 65536
M_P1 = 2049
N_CORES = 8
ROWS_PER_CORE = N_OBS // N_CORES  # 8192
RB = 128                          # rows per block
YPAD = 2052                       # y cols padded to a multiple of 4
YW = YPAD // 2                    # 1026 f32 words per padded row
SW = 2050                         # gather-source words: 1026 raw + 1024 shifted

_COMPILED = {}
LAST_RESULTS = None


def _host_consts(x: np.ndarray):
    """Nodes/interp-weights/basis from x, mirroring the jax reference."""
    import ml_dtypes

    bf16 = ml_dtypes.bfloat16
    x = np.asarray(x, dtype=np.float32)
    k = np.arange(DEG, dtype=np.float32)
    ang = (np.float32(np.pi) * (k + np.float32(0.5))) / np.float32(DEG)
    nodes = np.sort(np.cos(ang.astype(np.float32)).astype(np.float32))
    norm = ((np.float32(2.0) - (k == 0).astype(np.float32)) / np.float32(DEG)).astype(
        np.float64
    )
    theta = np.arccos(nodes.astype(np.float64))
    basis = norm[None, :] * np.cos(k.astype(np.float64)[None, :] * theta[:, None])
    idx = np.clip(np.searchsorted(x, nodes, side="right") - 1, 0, M_P1 - 2)
    a = x[idx]
    b = x[idx + 1]
    t = ((nodes - a) / (b - a)).astype(np.float64)

    # word index of the aligned pair (y[i], y[i+1]) in the gather source
    J = np.where(idx % 2 == 0, idx // 2, YW + (idx - 1) // 2).astype(np.int16)
    # wrapped for 16-partition gpsimd cores, replicated across the 8 cores
    Jw = np.tile(np.ascontiguousarray(J.reshape(64, 16).T), (8, 1))  # [128, 64]

    w2 = np.empty((1, DEG, 2), dtype=np.float64)
    w2[0, :, 0] = 1.0 - t
    w2[0, :, 1] = t
    return (
        np.ascontiguousarray(basis.astype(bf16)),      # [1024, 1024]
        np.ascontiguousarray(w2.astype(bf16)),         # [1, 1024, 2]
        np.ascontiguousarray(Jw),                      # [128, 64] int16
    )


def build_cheb_kernel(tc, y_ap, b_ap, w2_ap, j_ap, id_ap, o_ap, rows):
    import concourse.bass as bass
    import concourse.mybir as mybir

    nc = tc.nc
    f32 = mybir.dt.float32
    bf16 = mybir.dt.bfloat16
    nblocks = rows // RB

    with (
        tc.tile_pool(name="consts", bufs=1) as consts,
        tc.tile_pool(name="spool", bufs=3) as spool,
        tc.tile_pool(name="gpool", bufs=3) as gpool,
        tc.tile_pool(name="tpool", bufs=2) as tpool,
        tc.tile_pool(name="zpool", bufs=3) as zpool,
        tc.tile_pool(name="ztpool", bufs=2) as ztpool,
        tc.tile_pool(name="opool", bufs=3) as opool,
        tc.tile_pool(name="pst", bufs=2, space="PSUM") as pstp,
        tc.tile_pool(name="pso", bufs=2, space="PSUM") as psop,
    ):
        ident = consts.tile([128, 128], bf16)
        nc.sync.dma_start(out=ident, in_=id_ap)
        # basis resident in SBUF [part-within-chunk, chunk, d]; chunked DMAs
        # alternating queues so early matmuls aren't blocked on late chunks.
        b_sb = consts.tile([128, 8, DEG], bf16)
        b_r = b_ap.rearrange("(c p) n -> p c n", p=128)
        for c in range(8):
            eng = nc.scalar if c % 2 == 0 else nc.sync
            eng.dma_start(out=b_sb[:, c, :], in_=b_r[:, c, :])
        # lerp weights broadcast to all partitions
        w2_sb = consts.tile([128, DEG, 2], bf16)
        w2_bc = bass.AP(
            tensor=w2_ap.tensor, offset=w2_ap.offset,
            ap=[[0, 128]] + list(w2_ap.ap[1:]),
        )
        nc.scalar.dma_start(out=w2_sb, in_=w2_bc)
        jt = consts.tile([128, 64], mybir.dt.int16)
        nc.sync.dma_start(out=jt, in_=j_ap)

        state = {}

        def front(b):
            # DMA y block, build shifted copy, gather node pairs, lerp.
            s = spool.tile([128, SW], f32, name="s", tag="s")
            nc.sync.dma_start(out=s[:, 0:YW], in_=y_ap[b * RB : (b + 1) * RB, :])
            s16 = s.bitcast(bf16)
            nc.vector.tensor_copy(s16[:, 2 * YW : 2 * YW + 2048], s16[:, 1:2049])
            g = gpool.tile([128, DEG], f32, name="g", tag="g")
            nc.gpsimd.ap_gather(
                g, s, jt, channels=128, num_elems=SW, d=1, num_idxs=DEG
            )
            t = tpool.tile([128, DEG, 2], bf16, name="t", tag="t")
            g3 = g.bitcast(bf16).rearrange("p (d two) -> p d two", two=2)
            nc.vector.tensor_mul(t, g3, w2_sb)
            z = zpool.tile([128, DEG], bf16, name="z", tag="z")
            nc.vector.tensor_add(z, t[:, :, 0], t[:, :, 1])
            state[b] = z

        def back(b):
            z = state.pop(b)
            zt_ps = pstp.tile([128, 8, 128], bf16, name="ztp", tag="ztp")
            for j in range(8):
                nc.tensor.transpose(
                    zt_ps[:, j, :], z[:, j * 128 : (j + 1) * 128], ident
                )
            zt = ztpool.tile([128, 8, 128], bf16, name="zt", tag="zt")
            nc.scalar.copy(zt, zt_ps)
            ps = psop.tile([128, DEG], f32, name="ps", tag="ps")
            for c in range(8):
                for h in range(2):
                    nc.tensor.matmul(
                        ps[:, h * 512 : (h + 1) * 512],
                        zt[:, c, :],
                        b_sb[:, c, h * 512 : (h + 1) * 512],
                        start=(c == 0),
                        stop=(c == 7),
                    )
            osb = opool.tile([128, DEG], bf16, name="osb", tag="osb")
            nc.scalar.copy(osb, ps)
            nc.scalar.dma_start(out=o_ap[b * RB : (b + 1) * RB, :], in_=osb)

        for i in range(nblocks + 1):
            if i < nblocks:
                front(i)
            if i >= 1:
                back(i - 1)


def _build_nc(rows):
    import concourse.mybir as mybir
    import concourse.tile as tile
    from concourse import bacc

    f32 = mybir.dt.float32
    bf16 = mybir.dt.bfloat16
    nc = bacc.Bacc(
        "TRN2",
        target_bir_lowering=False,
        debug=False,
        enable_asserts=False,
        num_devices=N_CORES,
    )
    y_ap = nc.dram_tensor("y", [rows, YW], f32, kind="ExternalInput").ap()
    b_ap = nc.dram_tensor("b", [DEG, DEG], bf16, kind="ExternalInput").ap()
    w2_ap = nc.dram_tensor("w2", [1, DEG, 2], bf16, kind="ExternalInput").ap()
    j_ap = nc.dram_tensor("j", [128, 64], mybir.dt.int16, kind="ExternalInput").ap()
    id_ap = nc.dram_tensor("ident", [128, 128], bf16, kind="ExternalInput").ap()
    o_ap = nc.dram_tensor("o", [rows, DEG], bf16, kind="ExternalOutput").ap()
    with tile.TileContext(nc) as tc:
        build_cheb_kernel(tc, y_ap, b_ap, w2_ap, j_ap, id_ap, o_ap, rows)
    nc.compile()
    return nc


def _get_compiled(rows=ROWS_PER_CORE):
    if rows not in _COMPILED:
        _COMPILED[rows] = _build_nc(rows)
    return _COMPILED[rows]


def kernel(x: np.ndarray, y: np.ndarray) -> np.ndarray:
    global LAST_RESULTS
    import ml_dtypes
    from concourse import bass_utils

    bf16 = ml_dtypes.bfloat16
    x = np.asarray(x, dtype=np.float32)
    y = np.asarray(y, dtype=np.float32)
    assert y.shape == (N_OBS, M_P1), y.shape
    basis16, w2, Jw = _host_consts(x)

    y16 = np.zeros((N_OBS, YPAD), dtype=bf16)
    y16[:, :M_P1] = y.astype(bf16)
    yw = y16.view(np.float32)  # [N_OBS, YW]

    nc = _get_compiled()
    ident = np.ascontiguousarray(np.eye(128, dtype=np.float32).astype(bf16))
    in_maps = [
        {
            "y": yw[i * ROWS_PER_CORE : (i + 1) * ROWS_PER_CORE],
            "b": basis16,
            "w2": w2,
            "j": Jw,
            "ident": ident,
        }
        for i in range(N_CORES)
    ]
    trace = bool(int(os.environ.get("CHEB_TRACE", "0")))
    res = bass_utils.run_bass_kernel_spmd(
        nc, in_maps, core_ids=list(range(N_CORES)), trace=trace
    )
    LAST_RESULTS = res
    out = np.concatenate(
        [np.asarray(res.results[i]["o"]) for i in range(N_CORES)], axis=0
    )
    return out.astype(np.float32).reshape(-1)


# revision 6
# speedup vs baseline: 10369.4455x; 1.0117x over previous
"""Chebyshev approximation kernel for Trainium2 (8 NeuronCores, SPMD data-parallel).

Math: reference computes
    z   = (1-t) * y[:, idx] + t * y[:, idx+1]   # interp at nodes [n_obs, deg]
    out = (z @ basis).reshape(-1)               # [n_obs*deg]

Device pipeline (per 128-row block, y carried in bf16):
  1. DMA the y block (bf16, padded to 2052 cols) into SBUF.
  2. DVE copies a 1-element-shifted bf16 view next to it, so that for ANY
     index i the pair (y[i], y[i+1]) lives in a single aligned 4-byte word:
     even i -> word i/2 of the raw copy, odd i -> word (i-1)/2 of the
     shifted copy.
  3. GPSIMD ap_gather fetches one word per Chebyshev node (1024 idxs).
  4. DVE lerp: t = pairs * [1-t_d, t_d] (elementwise), z_d = t[2d]+t[2d+1].
  5. PE transposes z (8x bf16 128x128) and runs the k=1024 GEMM z @ basis
     (16 accumulating bf16 matmuls into PSUM).
  6. ACT drains PSUM; result written back as bf16 and upcast on host.

Interp weights/indices and the basis are computed on host from x (tiny).
Sharding: y rows split 8192/core across 8 cores; consts replicated.
"""

import os
import numpy as np

DEG = 1024
N_OBS = 65536
M_P1 = 2049
N_CORES = 8
ROWS_PER_CORE = N_OBS // N_CORES  # 8192
RB = 128                          # rows per block
YPAD = 2052                       # y cols padded to a multiple of 4
YW = YPAD // 2                    # 1026 f32 words per padded row
SW = 2050                         # gather-source words: 1026 raw + 1024 shifted

_COMPILED = {}
LAST_RESULTS = None


def _host_consts(x: np.ndarray):
    """Nodes/interp-weights/basis from x, mirroring the jax reference."""
    import ml_dtypes

    bf16 = ml_dtypes.bfloat16
    x = np.asarray(x, dtype=np.float32)
    k = np.arange(DEG, dtype=np.float32)
    ang = (np.float32(np.pi) * (k + np.float32(0.5))) / np.float32(DEG)
    nodes = np.sort(np.cos(ang.astype(np.float32)).astype(np.float32))
    norm = ((np.float32(2.0) - (k == 0).astype(np.float32)) / np.float32(DEG)).astype(
        np.float64
    )
    theta = np.arccos(nodes.astype(np.float64))
    basis = norm[None, :] * np.cos(k.astype(np.float64)[None, :] * theta[:, None])
    idx = np.clip(np.searchsorted(x, nodes, side="right") - 1, 0, M_P1 - 2)
    a = x[idx]
    b = x[idx + 1]
    t = ((nodes - a) / (b - a)).astype(np.float64)

    # word index of the aligned pair (y[i], y[i+1]) in the gather source
    J = np.where(idx % 2 == 0, idx // 2, YW + (idx - 1) // 2).astype(np.int16)
    # wrapped for 16-partition gpsimd cores, replicated across the 8 cores
    Jw = np.tile(np.ascontiguousarray(J.reshape(64, 16).T), (8, 1))  # [128, 64]

    w2 = np.empty((1, DEG, 2), dtype=np.float64)
    w2[0, :, 0] = 1.0 - t
    w2[0, :, 1] = t
    return (
        np.ascontiguousarray(basis.astype(bf16)),      # [1024, 1024]
        np.ascontiguousarray(w2.astype(bf16)),         # [1, 1024, 2]
        np.ascontiguousarray(Jw),                      # [128, 64] int16
    )


def build_cheb_kernel(tc, y_ap, b_ap, w2_ap, j_ap, id_ap, o_ap, rows):
    import concourse.bass as bass
    import concourse.mybir as mybir

    nc = tc.nc
    f32 = mybir.dt.float32
    bf16 = mybir.dt.bfloat16
    nblocks = rows // RB

    with (
        tc.tile_pool(name="consts", bufs=1) as consts,
        tc.tile_pool(name="spool", bufs=5) as spool,
        tc.tile_pool(name="gpool", bufs=3) as gpool,
        tc.tile_pool(name="tpool", bufs=2) as tpool,
        tc.tile_pool(name="zpool", bufs=3) as zpool,
        tc.tile_pool(name="ztpool", bufs=2) as ztpool,
        tc.tile_pool(name="opool", bufs=3) as opool,
        tc.tile_pool(name="pst", bufs=2, space="PSUM") as pstp,
        tc.tile_pool(name="pso", bufs=2, space="PSUM") as psop,
    ):
        ident = consts.tile([128, 128], bf16)
        nc.sync.dma_start(out=ident, in_=id_ap)
        # basis resident in SBUF [part-within-chunk, chunk, d]; chunked DMAs
        # alternating queues so early matmuls aren't blocked on late chunks.
        b_sb = consts.tile([128, 8, DEG], bf16)
        b_r = b_ap.rearrange("(c p) n -> p c n", p=128)
        for c in range(8):
            eng = nc.scalar if c % 2 == 0 else nc.sync
            eng.dma_start(out=b_sb[:, c, :], in_=b_r[:, c, :])
        # lerp weights broadcast to all partitions
        w2_sb = consts.tile([128, DEG, 2], bf16)
        w2_bc = bass.AP(
            tensor=w2_ap.tensor, offset=w2_ap.offset,
            ap=[[0, 128]] + list(w2_ap.ap[1:]),
        )
        nc.scalar.dma_start(out=w2_sb, in_=w2_bc)
        jt = consts.tile([128, 64], mybir.dt.int16)
        nc.sync.dma_start(out=jt, in_=j_ap)

        svec, gvec, state = {}, {}, {}

        def load(b):
            s = spool.tile([128, SW], f32, name="s", tag="s")
            nc.sync.dma_start(out=s[:, 0:YW], in_=y_ap[b * RB : (b + 1) * RB, :])
            svec[b] = s

        def shift(b):
            s16 = svec[b].bitcast(bf16)
            nc.vector.tensor_copy(s16[:, 2 * YW : 2 * YW + 2048], s16[:, 1:2049])

        def gather(b):
            g = gpool.tile([128, DEG], f32, name="g", tag="g")
            nc.gpsimd.ap_gather(
                g, svec.pop(b), jt, channels=128, num_elems=SW, d=1, num_idxs=DEG
            )
            gvec[b] = g

        def lerp(b):
            g = gvec.pop(b)
            t = tpool.tile([128, DEG, 2], bf16, name="t", tag="t")
            g3 = g.bitcast(bf16).rearrange("p (d two) -> p d two", two=2)
            nc.vector.tensor_mul(t, g3, w2_sb)
            z = zpool.tile([128, DEG], bf16, name="z", tag="z")
            nc.vector.tensor_add(z, t[:, :, 0], t[:, :, 1])
            state[b] = z

        def back(b):
            z = state.pop(b)
            zt_ps = pstp.tile([128, 8, 128], bf16, name="ztp", tag="ztp")
            for j in range(8):
                nc.tensor.transpose(
                    zt_ps[:, j, :], z[:, j * 128 : (j + 1) * 128], ident
                )
            zt = ztpool.tile([128, 8, 128], bf16, name="zt", tag="zt")
            nc.scalar.copy(zt, zt_ps)
            ps = psop.tile([128, DEG], f32, name="ps", tag="ps")
            for c in range(8):
                for h in range(2):
                    nc.tensor.matmul(
                        ps[:, h * 512 : (h + 1) * 512],
                        zt[:, c, :],
                        b_sb[:, c, h * 512 : (h + 1) * 512],
                        start=(c == 0),
                        stop=(c == 7),
                    )
            osb = opool.tile([128, DEG], bf16, name="osb", tag="osb")
            nc.scalar.copy(osb, ps)
            nc.scalar.dma_start(out=o_ap[b * RB : (b + 1) * RB, :], in_=osb)

        # Software pipeline with per-engine stage offsets so no engine's
        # in-order stream ever waits on a later stage of the same block:
        #   iter i: dma(i) | dve shift(i-2) | gp gather(i-3) | dve lerp(i-4)
        #           | pe/act/out back(i-5)
        for i in range(nblocks + 5):
            if i < nblocks:
                load(i)
            if 2 <= i < nblocks + 2:
                shift(i - 2)
            if 3 <= i < nblocks + 3:
                gather(i - 3)
            if 4 <= i < nblocks + 4:
                lerp(i - 4)
            if i >= 5:
                back(i - 5)


def _build_nc(rows):
    import concourse.mybir as mybir
    import concourse.tile as tile
    from concourse import bacc

    f32 = mybir.dt.float32
    bf16 = mybir.dt.bfloat16
    nc = bacc.Bacc(
        "TRN2",
        target_bir_lowering=False,
        debug=False,
        enable_asserts=False,
        num_devices=N_CORES,
    )
    y_ap = nc.dram_tensor("y", [rows, YW], f32, kind="ExternalInput").ap()
    b_ap = nc.dram_tensor("b", [DEG, DEG], bf16, kind="ExternalInput").ap()
    w2_ap = nc.dram_tensor("w2", [1, DEG, 2], bf16, kind="ExternalInput").ap()
    j_ap = nc.dram_tensor("j", [128, 64], mybir.dt.int16, kind="ExternalInput").ap()
    id_ap = nc.dram_tensor("ident", [128, 128], bf16, kind="ExternalInput").ap()
    o_ap = nc.dram_tensor("o", [rows, DEG], bf16, kind="ExternalOutput").ap()
    with tile.TileContext(nc) as tc:
        build_cheb_kernel(tc, y_ap, b_ap, w2_ap, j_ap, id_ap, o_ap, rows)
    nc.compile()
    return nc


def _get_compiled(rows=ROWS_PER_CORE):
    if rows not in _COMPILED:
        _COMPILED[rows] = _build_nc(rows)
    return _COMPILED[rows]


def kernel(x: np.ndarray, y: np.ndarray) -> np.ndarray:
    global LAST_RESULTS
    import ml_dtypes
    from concourse import bass_utils

    bf16 = ml_dtypes.bfloat16
    x = np.asarray(x, dtype=np.float32)
    y = np.asarray(y, dtype=np.float32)
    assert y.shape == (N_OBS, M_P1), y.shape
    basis16, w2, Jw = _host_consts(x)

    y16 = np.zeros((N_OBS, YPAD), dtype=bf16)
    y16[:, :M_P1] = y.astype(bf16)
    yw = y16.view(np.float32)  # [N_OBS, YW]

    nc = _get_compiled()
    ident = np.ascontiguousarray(np.eye(128, dtype=np.float32).astype(bf16))
    in_maps = [
        {
            "y": yw[i * ROWS_PER_CORE : (i + 1) * ROWS_PER_CORE],
            "b": basis16,
            "w2": w2,
            "j": Jw,
            "ident": ident,
        }
        for i in range(N_CORES)
    ]
    trace = bool(int(os.environ.get("CHEB_TRACE", "0")))
    res = bass_utils.run_bass_kernel_spmd(
        nc, in_maps, core_ids=list(range(N_CORES)), trace=trace
    )
    LAST_RESULTS = res
    out = np.concatenate(
        [np.asarray(res.results[i]["o"]) for i in range(N_CORES)], axis=0
    )
    return out.astype(np.float32).reshape(-1)
